# revision 21
# baseline (speedup 1.0000x reference)
"""Trainium2 Bass kernel for nn_Adaptive_Pooling_Layer (8-core data parallel).

Math (per batch, derived from the reference):
  x = mean_T(node_set)            [N=1024, D=64]   (X12 = 12*x kept unscaled)
  temp = mean_n x; h_avg = tanh(W0^T temp); att = x@h_avg; cent = att@x
  bc1 = relu(w_i2c cent^T + b_i2c);  bc2 = relu(W_lin bc1 + b_lin)  -> [H=8,K=128,D]
  cos[n,h,k] = <x_n, bc_hat_hk> / xn_n    (bc_hat = row-normalized bc2)
  S[n,h] = sum_k cos = <x_n, bsum_h>/xn_n   (bsum_h = sum_k bc_hat_hk)
  C[n,k] = sum_h conv_w[h]*cos/(S+eps) + conv_b
  new[k,d] = sum_n C[n,k] x[n,d];  out = new @ W_feat^T + b_feat

Key restructuring: with Mn[n,h] = conv_w12[h]*inv_xn12[n]/(S[n,h]+eps) and
y_h = Mn[:,h] * x12,  new^T = (1/144) * sum_h (X12^T @ y)_h^T @ bc_hat^T_h
so the [N x HK] cosine matrix is never materialized.

HW notes: PE matmuls with different lhsT partition bases must not target the
same PSUM bank (device fault).  fp32 matmuls emit 2 HW instructions; f32r
(TF32-like, ~1.5e-4 rel) emits 1 and runs 4x, used only on the output path
(z, newT) where the chaotic S-normalizer sensitivity doesn't apply.
"""
import os
import sys

import numpy as np

sys.path.insert(0, "/opt/trn_rl_repo")

B, Din, N, T = 32, 64, 1024, 12
H, K, Dout = 8, 128, 64
NCORES = 8
BPC = B // NCORES  # batches per core
NORM_EPS = 1e-10

_cache = {}

CFG = {"psA": 2, "psB": 1, "psZ": 1, "psS": 2, "nst": 1, "x12": 4,
       "xt2": 4, "bc": 1, "y": 3, "st": 2, "zs": 2, "split_tred": 1}


def _build(do_compile=True):
    import concourse.bass as bass
    import concourse.tile as tile
    from concourse import bacc, masks, mybir

    f32 = mybir.dt.float32
    f32r = mybir.dt.float32r
    AF = mybir.ActivationFunctionType
    ALU = mybir.AluOpType
    AX = mybir.AxisListType

    nc = bacc.Bacc("TRN2", target_bir_lowering=False, debug=False)

    node_d = nc.declare_dram_parameter("node_set", [BPC, Din, N, T], f32, isOutput=False)
    w0_d = nc.declare_dram_parameter("W0dup", [64, 128], f32, isOutput=False)
    i2c_d = nc.declare_dram_parameter("i2c_aug", [2, 8], f32, isOutput=False)
    lin_d = nc.declare_dram_parameter("lin_aug", [9, 1024], f32, isOutput=False)
    cw_d = nc.declare_dram_parameter("cw12", [128, 8], f32, isOutput=False)
    cb_d = nc.declare_dram_parameter("cb64", [64, 1], f32, isOutput=False)
    wf_d = nc.declare_dram_parameter("wf_aug", [65, 64], f32, isOutput=False)
    fold_d = nc.declare_dram_parameter("fold64", [128, 64], f32, isOutput=False)
    dup_d = nc.declare_dram_parameter("dup128", [64, 128], f32, isOutput=False)
    out_d = nc.declare_dram_parameter("out", [BPC, K, Dout], f32, isOutput=True)

    with tile.TileContext(nc) as tc:
        with (
            tc.tile_pool(name="const", bufs=1) as constp,
            tc.tile_pool(name="nst", bufs=CFG["nst"]) as nstp,
            tc.tile_pool(name="xt2", bufs=CFG["xt2"]) as xt2p,
            tc.tile_pool(name="x12", bufs=CFG["x12"]) as x12p,
            tc.tile_pool(name="bc", bufs=CFG["bc"]) as bcp,
            tc.tile_pool(name="y", bufs=CFG["y"]) as yp,
            tc.tile_pool(name="zs", bufs=CFG["zs"]) as zp,
            tc.tile_pool(name="st", bufs=CFG["st"]) as stp,
            tc.tile_pool(name="psA", bufs=CFG["psA"], space="PSUM") as psA,
            tc.tile_pool(name="psB", bufs=CFG["psB"], space="PSUM") as psB,
            tc.tile_pool(name="psZ", bufs=CFG["psZ"], space="PSUM") as psZ,
            tc.tile_pool(name="psS", bufs=CFG["psS"], space="PSUM") as psS,
        ):
            # ---- constants ----
            ident = constp.tile([128, 128], f32)
            masks.make_identity(nc, ident[:])
            w0_sb = constp.tile([64, 128], f32)
            nc.gpsimd.dma_start(out=w0_sb[:], in_=w0_d[:])
            i2c_sb = constp.tile([2, 8], f32)
            nc.gpsimd.dma_start(out=i2c_sb[:], in_=i2c_d[:])
            lin_sb = constp.tile([9, 1024], f32)
            nc.gpsimd.dma_start(out=lin_sb[:], in_=lin_d[:])
            cw_sb = constp.tile([128, 8], f32)
            nc.gpsimd.dma_start(out=cw_sb[:], in_=cw_d[:])
            cb_sb = constp.tile([64, 1], f32)
            nc.gpsimd.dma_start(out=cb_sb[:], in_=cb_d[:])
            wf_sb = constp.tile([65, 64], f32)
            nc.gpsimd.dma_start(out=wf_sb[:], in_=wf_d[:])
            fold_sb = constp.tile([128, 64], f32)
            nc.gpsimd.dma_start(out=fold_sb[:], in_=fold_d[:])
            dup_sb = constp.tile([64, 128], f32)
            nc.gpsimd.dma_start(out=dup_sb[:], in_=dup_d[:])
            eps_sb = constp.tile([128, 1], f32)
            nc.gpsimd.memset(eps_sb[:], 1e-30)

            nsts = []
            for b in range(B // NCORES):
                nst_b = nstp.tile([128, 6144], f32, tag=f"n{b}")
                for q in range(8):
                    p0, n0 = (0, 0) if q < 4 else (64, 512)
                    nq = n0 + (q % 4) * 128
                    f0 = (q % 4) * 1536
                    nc.sync.dma_start(
                        out=nst_b[p0:p0 + 64, f0:f0 + 1536]
                            .rearrange("p (n t) -> p n t", t=T),
                        in_=node_d[b, :, nq:nq + 128, :],
                    )
                nsts.append(nst_b)

            def xt2_slice(xT2, i):
                p0 = 0 if i < 4 else 64
                c0 = (i % 4) * 128
                return xT2[p0:p0 + 64, c0:c0 + 128]

            def half(t, i):
                return t[0:64, :] if i < 4 else t[64:128, :]

            def pair_stages(pr):
                # ================= T-sum (both batches) =================
                xT2s = []
                for v in range(2):
                    b = pr * 2 + v
                    nst = nsts[b]
                    xT2 = xt2p.tile([128, 512], f32)
                    for gg in range(2):
                        nc.vector.tensor_reduce(
                            xT2[:, gg * 256:(gg + 1) * 256],
                            nst[:, gg * 3072:(gg + 1) * 3072]
                                .rearrange("p (n t) -> p n t", t=T),
                            axis=AX.X, op=ALU.add,
                        )
                    xT2s.append(xT2)
                    yield

                # ============ transpose to x12 chunks, per batch ============
                x12s, x12rs = [], []
                xn2_2 = stp.tile([128, 16], f32)
                t128_2 = stp.tile([128, 2], f32)
                for v in range(2):
                    xT2 = xT2s[v]
                    xp1 = psA.tile([128, 256], f32, tag="xp1")
                    xp2 = psA.tile([128, 256], f32, tag="xp2")
                    for i in range(8):
                        idn = ident[0:64, 0:64] if i < 4 else ident[64:128, 64:128]
                        xp = xp1 if i < 4 else xp2
                        nc.tensor.transpose(
                            xp[:, (i % 4) * 64:(i % 4 + 1) * 64],
                            xt2_slice(xT2, i), idn,
                        )
                    x12 = x12p.tile([128, 512], f32)
                    nc.scalar.copy(x12[:, 0:256], xp1[:])
                    nc.scalar.copy(x12[:, 256:512], xp2[:])
                    x12r = x12p.tile([128, 512], f32r)
                    nc.scalar.copy(x12r[:, 0:256], xp1[:])
                    nc.scalar.copy(x12r[:, 256:512], xp2[:])
                    x12s.append(x12)
                    x12rs.append(x12r)
                    scr = stp.tile([128, 512], f32)
                    nc.scalar.activation(scr[:], x12[:], AF.Square)
                    nc.vector.tensor_reduce(
                        xn2_2[:, v * 8:(v + 1) * 8],
                        scr[:].rearrange("p (c d) -> p c d", c=8),
                        axis=AX.X, op=ALU.add,
                    )
                    nc.vector.tensor_reduce(
                        t128_2[:, v:v + 1], xT2[:], axis=AX.X, op=ALU.add,
                    )
                    if v == 0:
                        yield
                sq2 = stp.tile([128, 16], f32)
                nc.scalar.activation(sq2[:], xn2_2[:], AF.Sqrt, bias=eps_sb[:])
                inv12_2 = stp.tile([128, 16], f32)  # [n, (v, chunk)] 1/(12 xn)
                nc.vector.reciprocal(inv12_2[:], sq2[:])

                # ========== colsum / temp / h_avg (paired) ==========
                colsum2 = psS.tile([64, 2], f32, tag="small")
                nc.tensor.matmul(colsum2[:], fold_sb[:], t128_2[:], start=True, stop=True)
                tempT2 = stp.tile([64, 2], f32)
                nc.scalar.mul(tempT2[:], colsum2[:], 1.0 / (N * T))
                cbc2 = stp.tile([64, 2], f32)
                nc.scalar.mul(cbc2[:], colsum2[:], cb_sb[:])
                havg_ps2 = psS.tile([128, 2], f32, tag="small")
                nc.tensor.matmul(havg_ps2[:], w0_sb[:], tempT2[:], start=True, stop=True)
                havg2 = stp.tile([128, 2], f32)
                nc.scalar.activation(havg2[:], havg_ps2[:], AF.Tanh)

                # ================== att / cent, per batch ==================
                att2 = stp.tile([128, 16], f32)
                cent_pss = []
                for v in range(2):
                    xT2 = xT2s[v]
                    att_psA = psA.tile([128, 4], f32, tag="xp1")
                    att_psB = psA.tile([128, 4], f32, tag="xp2")
                    for i in range(8):
                        aps = att_psA if i < 4 else att_psB
                        nc.tensor.matmul(
                            aps[:, (i % 4):(i % 4) + 1], xt2_slice(xT2, i),
                            half(havg2, i)[:, v:v + 1], start=True, stop=True,
                        )
                    nc.scalar.copy(att2[:, v * 8:v * 8 + 4], att_psA[:])
                    nc.scalar.copy(att2[:, v * 8 + 4:v * 8 + 8], att_psB[:])
                    cent_ps = psS.tile([1, 64], f32, tag="small")
                    for i in range(8):
                        nc.tensor.matmul(
                            cent_ps[:], att2[:, v * 8 + i:v * 8 + i + 1],
                            x12s[v][:, i * 64:(i + 1) * 64],
                            start=(i == 0), stop=(i == 7),
                        )
                    cent_pss.append(cent_ps)
                yield

                # ============ centroid generator (paired) ============
                cent_aug2 = stp.tile([2, 128], f32)
                nc.gpsimd.memset(cent_aug2[:], 1.0)
                for v in range(2):
                    nc.scalar.mul(
                        cent_aug2[0:1, v * 64:(v + 1) * 64], cent_pss[v][:],
                        1.0 / 144.0,
                    )
                bc1_ps2 = psS.tile([8, 128], f32, tag="small")
                nc.tensor.matmul(bc1_ps2[:], i2c_sb[:], cent_aug2[:], start=True, stop=True)
                bc1_aug2 = stp.tile([9, 128], f32)
                nc.gpsimd.memset(bc1_aug2[:], 1.0)
                nc.scalar.activation(bc1_aug2[0:8, :], bc1_ps2[:], AF.Relu)

                # bcr2: [k, (v, h, d)] relu'd bc2 for both batches
                bcr2 = bcp.tile([128, 1024], f32)
                bcr2_v = bcr2[:].rearrange("p (v h d) -> p v h d", v=2, h=8)
                for h in range(8):
                    bc2_ps = psS.tile([128, 128], f32, tag="small")
                    nc.tensor.matmul(
                        bc2_ps[:], lin_sb[:, h * 128:(h + 1) * 128], bc1_aug2[:],
                        start=True, stop=True,
                    )
                    nc.scalar.activation(
                        bcr2_v[:, :, h, :],
                        bc2_ps[:].rearrange("p (v d) -> p v d", v=2), AF.Relu,
                    )
                scrc = stp.tile([128, 1024], f32)
                nc.vector.scalar_tensor_tensor(
                    out=scrc[:], in0=bcr2[:], scalar=0.0, in1=bcr2[:],
                    op0=ALU.bypass, op1=ALU.mult,
                )
                cn2_2 = stp.tile([128, 16], f32)
                nc.vector.tensor_reduce(
                    cn2_2[:], scrc[:].rearrange("p (g d) -> p g d", g=16),
                    axis=AX.X, op=ALU.add,
                )
                sqc2 = stp.tile([128, 16], f32)
                nc.scalar.activation(sqc2[:], cn2_2[:], AF.Sqrt, bias=eps_sb[:])
                invcn2 = stp.tile([128, 16], f32)
                nc.vector.reciprocal(invcn2[:], sqc2[:])
                bchat2 = bcp.tile([128, 1024], f32)  # [k, (v, h, d)] normalized
                nc.vector.scalar_tensor_tensor(
                    out=bchat2[:].rearrange("p (g d) -> p g d", g=16),
                    in0=bcr2[:].rearrange("p (g d) -> p g d", g=16),
                    scalar=0.0,
                    in1=invcn2[:].rearrange("p (g d) -> p g d", d=1)
                        .broadcast_to([128, 16, 64]),
                    op0=ALU.bypass, op1=ALU.mult,
                )

                # ====== bc_hat transposes -> bchT2 [d, (v, h, k)] ======
                bchT2 = bcp.tile([64, 2048], f32)
                bchTr2 = bcp.tile([64, 2048], f32r)
                for g in range(4):
                    bchT_ps = psB.tile([64, 512], f32, tag="bchT")
                    for j in range(4):
                        hh = g * 4 + j  # global (v, h) index
                        nc.tensor.transpose(
                            bchT_ps[:, j * 128:(j + 1) * 128],
                            bchat2[:, hh * 64:(hh + 1) * 64], ident[:],
                        )
                    nc.scalar.copy(bchT2[:, g * 512:(g + 1) * 512], bchT_ps[:])
                    nc.scalar.copy(bchTr2[:, g * 512:(g + 1) * 512], bchT_ps[:])

                # ====== bsum (both batches) + duplication matmul ======
                bsum2 = stp.tile([64, 16], f32)
                nc.vector.tensor_reduce(
                    bsum2[:], bchT2[:].rearrange("p (g k) -> p g k", g=16),
                    axis=AX.X, op=ALU.add,
                )
                bsum_ps2 = psS.tile([128, 16], f32, tag="small")
                nc.tensor.matmul(bsum_ps2[:], dup_sb[:], bsum2[:], start=True, stop=True)
                bsumT2 = stp.tile([128, 16], f32)  # [(dup), (v, h)]
                nc.scalar.copy(bsumT2[:], bsum_ps2[:])
                yield

                # ============ S, E/Mn, y, z, newT, out per batch ============
                for v in range(2):
                    b = pr * 2 + v
                    xT2 = xT2s[v]
                    x12 = x12s[v]
                    x12r = x12rs[v]
                    S_A = psA.tile([128, 32], f32, tag="xp1")
                    S_B = psA.tile([128, 32], f32, tag="xp2")
                    for i in range(8):
                        sp = S_A if i < 4 else S_B
                        nc.tensor.matmul(
                            sp[:, (i % 4) * 8:(i % 4 + 1) * 8], xt2_slice(xT2, i),
                            half(bsumT2, i)[:, v * 8:(v + 1) * 8],
                            start=True, stop=True,
                        )
                    Mn = stp.tile([128, 64], f32)  # [n, (chunk, h)]
                    for g, sp in enumerate((S_A, S_B)):
                        ms = Mn[:, g * 32:(g + 1) * 32]
                        iv = (
                            inv12_2[:, v * 8 + g * 4:v * 8 + (g + 1) * 4]
                            .rearrange("p (c h) -> p c h", h=1)
                            .broadcast_to([128, 4, 8])
                        )
                        msv = ms.rearrange("p (c h) -> p c h", c=4)
                        nc.vector.scalar_tensor_tensor(
                            out=msv, in0=sp[:].rearrange("p (c h) -> p c h", c=4),
                            scalar=0.0, in1=iv, op0=ALU.bypass, op1=ALU.mult,
                        )
                        nc.vector.tensor_scalar(
                            out=ms, in0=ms, scalar1=NORM_EPS, scalar2=None,
                            op0=ALU.add,
                        )
                        nc.vector.reciprocal(ms, ms)
                        nc.vector.scalar_tensor_tensor(
                            out=msv, in0=msv, scalar=0.0, in1=iv,
                            op0=ALU.bypass, op1=ALU.mult,
                        )
                        nc.vector.scalar_tensor_tensor(
                            out=msv, in0=msv, scalar=0.0,
                            in1=cw_sb[:].rearrange("p (c h) -> p c h", c=1)
                                .broadcast_to([128, 4, 8]),
                            op0=ALU.bypass, op1=ALU.mult,
                        )

                    z_ps = psZ.tile([64, 512], f32, tag="zz")
                    for i in range(8):
                        yt = yp.tile([128, 512], f32r)
                        nc.vector.scalar_tensor_tensor(
                            out=yt[:].rearrange("p (h d) -> p h d", h=8),
                            in0=x12[:, i * 64:(i + 1) * 64]
                                .rearrange("p (h d) -> p h d", h=1)
                                .broadcast_to([128, 8, 64]),
                            scalar=0.0,
                            in1=Mn[:, i * 8:(i + 1) * 8]
                                .rearrange("p (h d) -> p h d", d=1)
                                .broadcast_to([128, 8, 64]),
                            op0=ALU.bypass, op1=ALU.mult,
                        )
                        nc.tensor.matmul(
                            z_ps[:], x12r[:, i * 64:(i + 1) * 64], yt[:],
                            start=(i == 0), stop=(i == 7),
                        )
                    z_sb = zp.tile([64, 512], f32r)
                    nc.scalar.mul(z_sb[:], z_ps[:], 1.0 / 144.0)

                    newT_ps = psZ.tile([64, 128], f32, tag="zz")
                    for h in range(8):
                        nc.tensor.matmul(
                            newT_ps[:],
                            z_sb[:, h * 64:(h + 1) * 64],
                            bchTr2[:, (v * 8 + h) * 128:(v * 8 + h + 1) * 128],
                            start=(h == 0), stop=(h == 7),
                        )
                    newT_aug = stp.tile([65, 128], f32)
                    nc.gpsimd.memset(newT_aug[:], 1.0)
                    nc.vector.tensor_scalar(
                        out=newT_aug[0:64, :], in0=newT_ps[:],
                        scalar1=cbc2[:, v:v + 1], scalar2=None, op0=ALU.add,
                    )
                    fin_ps = psS.tile([128, 64], f32, tag="small")
                    nc.tensor.matmul(fin_ps[:], newT_aug[:], wf_sb[:],
                                     start=True, stop=True)
                    out_sb = stp.tile([128, 64], f32)
                    nc.scalar.copy(out_sb[:], fin_ps[:])
                    nc.sync.dma_start(out=out_d[b], in_=out_sb[:])
                    if v == 0:
                        yield
                yield

            # drive (finer): overlap pair1 stages into pair0's tail
            g0 = pair_stages(0)
            g1 = pair_stages(1)
            next(g0)           # R0_0
            next(g0)           # R1_0
            next(g0)           # A0_0
            next(g0)           # A1_0 (+sq/colsum/havg/att/cent)
            next(g0)           # B_0
            next(g1)           # R0_1
            next(g0)           # C0_0 (v0 out)
            next(g1)           # R1_1
            next(g0, None)     # C1_0 (v1 out, end)
            next(g1)           # A0_1
            next(g1)           # A1_1
            next(g1)           # B_1
            next(g1)           # C0_1
            next(g1, None)     # C1_1

    if do_compile:
        nc.compile()
    else:
        nc.insert_bir_kernel_barrier_sem_inc()
    return nc


def _get_nc(do_compile=True):
    key = "nc" if do_compile else "nc_sim"
    if key not in _cache:
        _cache[key] = _build(do_compile)
    return _cache[key]


def prepare(inputs, do_compile=True):
    """Build (nc, in_maps) — shared by kernel() and the local sim harness."""
    node_set = np.asarray(inputs["node_set"], dtype=np.float32)
    W0 = np.asarray(inputs["W0"], dtype=np.float32)
    w_i2c = np.asarray(inputs["w_i2c"], dtype=np.float32)
    b_i2c = np.asarray(inputs["b_i2c"], dtype=np.float32)
    W_lin = np.asarray(inputs["W_lin"], dtype=np.float32)
    b_lin = np.asarray(inputs["b_lin"], dtype=np.float32)
    conv_w = np.asarray(inputs["conv_w"], dtype=np.float32)
    conv_b = np.asarray(inputs["conv_b"], dtype=np.float32)
    W_feat = np.asarray(inputs["W_feat"], dtype=np.float32)
    b_feat = np.asarray(inputs["b_feat"], dtype=np.float32)

    i2c_aug = np.ascontiguousarray(
        np.concatenate([w_i2c.T, b_i2c[None, :]], axis=0), dtype=np.float32
    )
    lin_aug = np.ascontiguousarray(
        np.concatenate([W_lin.T, b_lin[None, :]], axis=0), dtype=np.float32
    )
    w0dup = np.ascontiguousarray(np.concatenate([W0, W0], axis=1), dtype=np.float32)
    cw12 = np.ascontiguousarray(np.tile((12.0 * conv_w)[None, :], (128, 1)), dtype=np.float32)
    cb64 = np.full((64, 1), float(conv_b) / 12.0, dtype=np.float32)
    wf_aug = np.ascontiguousarray(
        np.concatenate([W_feat.T, b_feat[None, :]], axis=0), dtype=np.float32
    )
    pp = np.arange(128)
    mm = np.arange(64)
    fold64 = (pp[:, None] % 64 == mm[None, :]).astype(np.float32)
    dup128 = (np.arange(128)[None, :] % 64 == np.arange(64)[:, None]).astype(np.float32)

    nc = _get_nc(do_compile)
    in_maps = []
    for c in range(NCORES):
        in_maps.append({
            "node_set": np.ascontiguousarray(node_set[c * BPC:(c + 1) * BPC]),
            "W0dup": w0dup,
            "i2c_aug": i2c_aug,
            "lin_aug": lin_aug,
            "cw12": cw12,
            "cb64": cb64,
            "wf_aug": wf_aug,
            "fold64": fold64,
            "dup128": dup128,
        })
    return nc, in_maps


def kernel(**inputs):
    from concourse.bass_utils import run_bass_kernel_spmd

    nc, in_maps = prepare(inputs)

    res = run_bass_kernel_spmd(
        nc, in_maps, core_ids=list(range(NCORES)),
        trace=bool(os.environ.get("BASS_TRACE")),
    )
    _cache["last_results"] = res
    out = np.concatenate([res.results[i]["out"] for i in range(NCORES)], axis=0)
    return out



# revision 22
# speedup vs baseline: 1.0002x; 1.0002x over previous
"""Trainium2 Bass kernel for nn_Adaptive_Pooling_Layer (8-core data parallel).

Math (per batch, derived from the reference):
  x = mean_T(node_set)            [N=1024, D=64]   (X12 = 12*x kept unscaled)
  temp = mean_n x; h_avg = tanh(W0^T temp); att = x@h_avg; cent = att@x
  bc1 = relu(w_i2c cent^T + b_i2c);  bc2 = relu(W_lin bc1 + b_lin)  -> [H=8,K=128,D]
  cos[n,h,k] = <x_n, bc_hat_hk> / xn_n    (bc_hat = row-normalized bc2)
  S[n,h] = sum_k cos = <x_n, bsum_h>/xn_n   (bsum_h = sum_k bc_hat_hk)
  C[n,k] = sum_h conv_w[h]*cos/(S+eps) + conv_b
  new[k,d] = sum_n C[n,k] x[n,d];  out = new @ W_feat^T + b_feat

Key restructuring: with Mn[n,h] = conv_w12[h]*inv_xn12[n]/(S[n,h]+eps) and
y_h = Mn[:,h] * x12,  new^T = (1/144) * sum_h (X12^T @ y)_h^T @ bc_hat^T_h
so the [N x HK] cosine matrix is never materialized.

HW notes: PE matmuls with different lhsT partition bases must not target the
same PSUM bank (device fault).  fp32 matmuls emit 2 HW instructions; f32r
(TF32-like, ~1.5e-4 rel) emits 1 and runs 4x, used only on the output path
(z, newT) where the chaotic S-normalizer sensitivity doesn't apply.
"""
import os
import sys

import numpy as np

sys.path.insert(0, "/opt/trn_rl_repo")

B, Din, N, T = 32, 64, 1024, 12
H, K, Dout = 8, 128, 64
NCORES = 8
BPC = B // NCORES  # batches per core
NORM_EPS = 1e-10

_cache = {}

CFG = {"psA": 1, "psB": 1, "psZ": 2, "psS": 3, "nst": 1, "x12": 4,
       "xt2": 4, "bc": 1, "y": 3, "st": 2, "zs": 2, "split_tred": 1}


def _build(do_compile=True):
    import concourse.bass as bass
    import concourse.tile as tile
    from concourse import bacc, masks, mybir

    f32 = mybir.dt.float32
    f32r = mybir.dt.float32r
    AF = mybir.ActivationFunctionType
    ALU = mybir.AluOpType
    AX = mybir.AxisListType

    nc = bacc.Bacc("TRN2", target_bir_lowering=False, debug=False)

    node_d = nc.declare_dram_parameter("node_set", [BPC, Din, N, T], f32, isOutput=False)
    w0_d = nc.declare_dram_parameter("W0dup", [64, 128], f32, isOutput=False)
    i2c_d = nc.declare_dram_parameter("i2c_aug", [2, 8], f32, isOutput=False)
    lin_d = nc.declare_dram_parameter("lin_aug", [9, 1024], f32, isOutput=False)
    cw_d = nc.declare_dram_parameter("cw12", [128, 8], f32, isOutput=False)
    cb_d = nc.declare_dram_parameter("cb64", [64, 1], f32, isOutput=False)
    wf_d = nc.declare_dram_parameter("wf_aug", [65, 64], f32, isOutput=False)
    fold_d = nc.declare_dram_parameter("fold64", [128, 64], f32, isOutput=False)
    dup_d = nc.declare_dram_parameter("dup128", [64, 128], f32, isOutput=False)
    out_d = nc.declare_dram_parameter("out", [BPC, K, Dout], f32, isOutput=True)

    with tile.TileContext(nc) as tc:
        with (
            tc.tile_pool(name="const", bufs=1) as constp,
            tc.tile_pool(name="nst", bufs=CFG["nst"]) as nstp,
            tc.tile_pool(name="xt2", bufs=CFG["xt2"]) as xt2p,
            tc.tile_pool(name="x12", bufs=CFG["x12"]) as x12p,
            tc.tile_pool(name="bc", bufs=CFG["bc"]) as bcp,
            tc.tile_pool(name="y", bufs=CFG["y"]) as yp,
            tc.tile_pool(name="zs", bufs=CFG["zs"]) as zp,
            tc.tile_pool(name="st", bufs=CFG["st"]) as stp,
            tc.tile_pool(name="psA", bufs=CFG["psA"], space="PSUM") as psA,
            tc.tile_pool(name="psB", bufs=CFG["psB"], space="PSUM") as psB,
            tc.tile_pool(name="psZ", bufs=CFG["psZ"], space="PSUM") as psZ,
            tc.tile_pool(name="psS", bufs=CFG["psS"], space="PSUM") as psS,
        ):
            # ---- constants ----
            ident = constp.tile([128, 128], f32)
            masks.make_identity(nc, ident[:])
            w0_sb = constp.tile([64, 128], f32)
            nc.gpsimd.dma_start(out=w0_sb[:], in_=w0_d[:])
            i2c_sb = constp.tile([2, 8], f32)
            nc.gpsimd.dma_start(out=i2c_sb[:], in_=i2c_d[:])
            lin_sb = constp.tile([9, 1024], f32)
            nc.gpsimd.dma_start(out=lin_sb[:], in_=lin_d[:])
            cw_sb = constp.tile([128, 8], f32)
            nc.gpsimd.dma_start(out=cw_sb[:], in_=cw_d[:])
            cb_sb = constp.tile([64, 1], f32)
            nc.gpsimd.dma_start(out=cb_sb[:], in_=cb_d[:])
            wf_sb = constp.tile([65, 64], f32)
            nc.gpsimd.dma_start(out=wf_sb[:], in_=wf_d[:])
            fold_sb = constp.tile([128, 64], f32)
            nc.gpsimd.dma_start(out=fold_sb[:], in_=fold_d[:])
            dup_sb = constp.tile([64, 128], f32)
            nc.gpsimd.dma_start(out=dup_sb[:], in_=dup_d[:])
            eps_sb = constp.tile([128, 1], f32)
            nc.gpsimd.memset(eps_sb[:], 1e-30)

            nsts = []
            for b in range(B // NCORES):
                nst_b = nstp.tile([128, 6144], f32, tag=f"n{b}")
                for q in range(4):
                    p0, n0 = (0, 0) if q < 2 else (64, 512)
                    nq = n0 + (q % 2) * 256
                    nc.sync.dma_start(
                        out=nst_b[p0:p0 + 64, (q % 2) * 3072:(q % 2) * 3072 + 3072]
                            .rearrange("p (n t) -> p n t", t=T),
                        in_=node_d[b, :, nq:nq + 256, :],
                    )
                nsts.append(nst_b)

            def xt2_slice(xT2, i):
                p0 = 0 if i < 4 else 64
                c0 = (i % 4) * 128
                return xT2[p0:p0 + 64, c0:c0 + 128]

            def half(t, i):
                return t[0:64, :] if i < 4 else t[64:128, :]

            def pair_stages(pr):
                # ================= T-sum (both batches) =================
                xT2s = []
                for v in range(2):
                    b = pr * 2 + v
                    nst = nsts[b]
                    xT2 = xt2p.tile([128, 512], f32)
                    for gg in range(2):
                        nc.vector.tensor_reduce(
                            xT2[:, gg * 256:(gg + 1) * 256],
                            nst[:, gg * 3072:(gg + 1) * 3072]
                                .rearrange("p (n t) -> p n t", t=T),
                            axis=AX.X, op=ALU.add,
                        )
                    xT2s.append(xT2)
                    yield

                # ============ transpose to x12 chunks, per batch ============
                x12s, x12rs = [], []
                xn2_2 = stp.tile([128, 16], f32)
                t128_2 = stp.tile([128, 2], f32)
                for v in range(2):
                    xT2 = xT2s[v]
                    xp1 = psA.tile([128, 256], f32, tag="xp1")
                    xp2 = psA.tile([128, 256], f32, tag="xp2")
                    for i in range(8):
                        idn = ident[0:64, 0:64] if i < 4 else ident[64:128, 64:128]
                        xp = xp1 if i < 4 else xp2
                        nc.tensor.transpose(
                            xp[:, (i % 4) * 64:(i % 4 + 1) * 64],
                            xt2_slice(xT2, i), idn,
                        )
                    x12 = x12p.tile([128, 512], f32)
                    nc.scalar.copy(x12[:, 0:256], xp1[:])
                    nc.scalar.copy(x12[:, 256:512], xp2[:])
                    x12r = x12p.tile([128, 512], f32r)
                    nc.scalar.copy(x12r[:, 0:256], xp1[:])
                    nc.scalar.copy(x12r[:, 256:512], xp2[:])
                    x12s.append(x12)
                    x12rs.append(x12r)
                    scr = stp.tile([128, 512], f32)
                    nc.scalar.activation(scr[:], x12[:], AF.Square)
                    nc.vector.tensor_reduce(
                        xn2_2[:, v * 8:(v + 1) * 8],
                        scr[:].rearrange("p (c d) -> p c d", c=8),
                        axis=AX.X, op=ALU.add,
                    )
                    nc.vector.tensor_reduce(
                        t128_2[:, v:v + 1], xT2[:], axis=AX.X, op=ALU.add,
                    )
                    if v == 0:
                        yield
                sq2 = stp.tile([128, 16], f32)
                nc.scalar.activation(sq2[:], xn2_2[:], AF.Sqrt, bias=eps_sb[:])
                inv12_2 = stp.tile([128, 16], f32)  # [n, (v, chunk)] 1/(12 xn)
                nc.vector.reciprocal(inv12_2[:], sq2[:])

                # ========== colsum / temp / h_avg (paired) ==========
                colsum2 = psS.tile([64, 2], f32, tag="small")
                nc.tensor.matmul(colsum2[:], fold_sb[:], t128_2[:], start=True, stop=True)
                tempT2 = stp.tile([64, 2], f32)
                nc.scalar.mul(tempT2[:], colsum2[:], 1.0 / (N * T))
                cbc2 = stp.tile([64, 2], f32)
                nc.scalar.mul(cbc2[:], colsum2[:], cb_sb[:])
                havg_ps2 = psS.tile([128, 2], f32, tag="small")
                nc.tensor.matmul(havg_ps2[:], w0_sb[:], tempT2[:], start=True, stop=True)
                havg2 = stp.tile([128, 2], f32)
                nc.scalar.activation(havg2[:], havg_ps2[:], AF.Tanh)

                # ================== att / cent, per batch ==================
                att2 = stp.tile([128, 16], f32)
                cent_pss = []
                for v in range(2):
                    xT2 = xT2s[v]
                    att_psA = psA.tile([128, 4], f32, tag="xp1")
                    att_psB = psA.tile([128, 4], f32, tag="xp2")
                    for i in range(8):
                        aps = att_psA if i < 4 else att_psB
                        nc.tensor.matmul(
                            aps[:, (i % 4):(i % 4) + 1], xt2_slice(xT2, i),
                            half(havg2, i)[:, v:v + 1], start=True, stop=True,
                        )
                    nc.scalar.copy(att2[:, v * 8:v * 8 + 4], att_psA[:])
                    nc.scalar.copy(att2[:, v * 8 + 4:v * 8 + 8], att_psB[:])
                    cent_ps = psS.tile([1, 64], f32, tag="small")
                    for i in range(8):
                        nc.tensor.matmul(
                            cent_ps[:], att2[:, v * 8 + i:v * 8 + i + 1],
                            x12s[v][:, i * 64:(i + 1) * 64],
                            start=(i == 0), stop=(i == 7),
                        )
                    cent_pss.append(cent_ps)
                yield

                # ============ centroid generator (paired) ============
                cent_aug2 = stp.tile([2, 128], f32)
                nc.gpsimd.memset(cent_aug2[:], 1.0)
                for v in range(2):
                    nc.scalar.mul(
                        cent_aug2[0:1, v * 64:(v + 1) * 64], cent_pss[v][:],
                        1.0 / 144.0,
                    )
                bc1_ps2 = psS.tile([8, 128], f32, tag="small")
                nc.tensor.matmul(bc1_ps2[:], i2c_sb[:], cent_aug2[:], start=True, stop=True)
                bc1_aug2 = stp.tile([9, 128], f32)
                nc.gpsimd.memset(bc1_aug2[:], 1.0)
                nc.scalar.activation(bc1_aug2[0:8, :], bc1_ps2[:], AF.Relu)

                # bcr2: [k, (v, h, d)] relu'd bc2 for both batches
                bcr2 = bcp.tile([128, 1024], f32)
                bcr2_v = bcr2[:].rearrange("p (v h d) -> p v h d", v=2, h=8)
                for h in range(8):
                    bc2_ps = psS.tile([128, 128], f32, tag="small")
                    nc.tensor.matmul(
                        bc2_ps[:], lin_sb[:, h * 128:(h + 1) * 128], bc1_aug2[:],
                        start=True, stop=True,
                    )
                    nc.scalar.activation(
                        bcr2_v[:, :, h, :],
                        bc2_ps[:].rearrange("p (v d) -> p v d", v=2), AF.Relu,
                    )
                scrc = stp.tile([128, 1024], f32)
                nc.vector.scalar_tensor_tensor(
                    out=scrc[:], in0=bcr2[:], scalar=0.0, in1=bcr2[:],
                    op0=ALU.bypass, op1=ALU.mult,
                )
                cn2_2 = stp.tile([128, 16], f32)
                nc.vector.tensor_reduce(
                    cn2_2[:], scrc[:].rearrange("p (g d) -> p g d", g=16),
                    axis=AX.X, op=ALU.add,
                )
                sqc2 = stp.tile([128, 16], f32)
                nc.scalar.activation(sqc2[:], cn2_2[:], AF.Sqrt, bias=eps_sb[:])
                invcn2 = stp.tile([128, 16], f32)
                nc.vector.reciprocal(invcn2[:], sqc2[:])
                bchat2 = bcp.tile([128, 1024], f32)  # [k, (v, h, d)] normalized
                nc.vector.scalar_tensor_tensor(
                    out=bchat2[:].rearrange("p (g d) -> p g d", g=16),
                    in0=bcr2[:].rearrange("p (g d) -> p g d", g=16),
                    scalar=0.0,
                    in1=invcn2[:].rearrange("p (g d) -> p g d", d=1)
                        .broadcast_to([128, 16, 64]),
                    op0=ALU.bypass, op1=ALU.mult,
                )

                # ====== bc_hat transposes -> bchTr2 [d, (v, h, k)] ======
                # bsum reduced straight from each PSUM group (same k-order,
                # same values as reducing the SBUF copy).
                bchTr2 = bcp.tile([64, 2048], f32r)
                bsum2 = stp.tile([64, 16], f32)
                for g in range(4):
                    bchT_ps = psB.tile([64, 512], f32, tag="bchT")
                    for j in range(4):
                        hh = g * 4 + j  # global (v, h) index
                        nc.tensor.transpose(
                            bchT_ps[:, j * 128:(j + 1) * 128],
                            bchat2[:, hh * 64:(hh + 1) * 64], ident[:],
                        )
                    nc.scalar.copy(bchTr2[:, g * 512:(g + 1) * 512], bchT_ps[:])
                    nc.vector.tensor_reduce(
                        bsum2[:, g * 4:(g + 1) * 4],
                        bchT_ps[:].rearrange("p (g k) -> p g k", g=4),
                        axis=AX.X, op=ALU.add,
                    )
                bsum_ps2 = psS.tile([128, 16], f32, tag="small")
                nc.tensor.matmul(bsum_ps2[:], dup_sb[:], bsum2[:], start=True, stop=True)
                bsumT2 = stp.tile([128, 16], f32)  # [(dup), (v, h)]
                nc.scalar.copy(bsumT2[:], bsum_ps2[:])
                yield

                # ============ S, E/Mn, y, z, newT, out per batch ============
                for v in range(2):
                    b = pr * 2 + v
                    xT2 = xT2s[v]
                    x12 = x12s[v]
                    x12r = x12rs[v]
                    S_A = psA.tile([128, 32], f32, tag="xp1")
                    S_B = psA.tile([128, 32], f32, tag="xp2")
                    for i in range(8):
                        sp = S_A if i < 4 else S_B
                        nc.tensor.matmul(
                            sp[:, (i % 4) * 8:(i % 4 + 1) * 8], xt2_slice(xT2, i),
                            half(bsumT2, i)[:, v * 8:(v + 1) * 8],
                            start=True, stop=True,
                        )
                    S_sb = stp.tile([128, 64], f32)
                    nc.scalar.copy(S_sb[:, 0:32], S_A[:])
                    nc.scalar.copy(S_sb[:, 32:64], S_B[:])
                    Mn = stp.tile([128, 64], f32)  # [n, (chunk, h)]
                    iv = (
                        inv12_2[:, v * 8:(v + 1) * 8]
                        .rearrange("p (c h) -> p c h", h=1)
                        .broadcast_to([128, 8, 8])
                    )
                    msv = Mn[:].rearrange("p (c h) -> p c h", c=8)
                    nc.vector.scalar_tensor_tensor(
                        out=msv, in0=S_sb[:].rearrange("p (c h) -> p c h", c=8),
                        scalar=0.0, in1=iv, op0=ALU.bypass, op1=ALU.mult,
                    )
                    nc.vector.tensor_scalar(
                        out=Mn[:], in0=Mn[:], scalar1=NORM_EPS, scalar2=None,
                        op0=ALU.add,
                    )
                    nc.vector.reciprocal(Mn[:], Mn[:])
                    nc.vector.scalar_tensor_tensor(
                        out=msv, in0=msv, scalar=0.0, in1=iv,
                        op0=ALU.bypass, op1=ALU.mult,
                    )
                    nc.vector.scalar_tensor_tensor(
                        out=msv, in0=msv, scalar=0.0,
                        in1=cw_sb[:].rearrange("p (c h) -> p c h", c=1)
                            .broadcast_to([128, 8, 8]),
                        op0=ALU.bypass, op1=ALU.mult,
                    )

                    z_ps = psZ.tile([64, 512], f32, tag="zz")
                    for i in range(8):
                        yt = yp.tile([128, 512], f32r)
                        nc.vector.scalar_tensor_tensor(
                            out=yt[:].rearrange("p (h d) -> p h d", h=8),
                            in0=x12[:, i * 64:(i + 1) * 64]
                                .rearrange("p (h d) -> p h d", h=1)
                                .broadcast_to([128, 8, 64]),
                            scalar=0.0,
                            in1=Mn[:, i * 8:(i + 1) * 8]
                                .rearrange("p (h d) -> p h d", d=1)
                                .broadcast_to([128, 8, 64]),
                            op0=ALU.bypass, op1=ALU.mult,
                        )
                        nc.tensor.matmul(
                            z_ps[:], x12r[:, i * 64:(i + 1) * 64], yt[:],
                            start=(i == 0), stop=(i == 7),
                        )
                    z_sb = zp.tile([64, 512], f32r)
                    nc.scalar.mul(z_sb[:], z_ps[:], 1.0 / 144.0)

                    newT_ps = psZ.tile([64, 128], f32, tag="zz")
                    for h in range(8):
                        nc.tensor.matmul(
                            newT_ps[:],
                            z_sb[:, h * 64:(h + 1) * 64],
                            bchTr2[:, (v * 8 + h) * 128:(v * 8 + h + 1) * 128],
                            start=(h == 0), stop=(h == 7),
                        )
                    newT_aug = stp.tile([65, 128], f32)
                    nc.gpsimd.memset(newT_aug[:], 1.0)
                    nc.vector.tensor_scalar(
                        out=newT_aug[0:64, :], in0=newT_ps[:],
                        scalar1=cbc2[:, v:v + 1], scalar2=None, op0=ALU.add,
                    )
                    fin_ps = psS.tile([128, 64], f32, tag="small")
                    nc.tensor.matmul(fin_ps[:], newT_aug[:], wf_sb[:],
                                     start=True, stop=True)
                    out_sb = stp.tile([128, 64], f32)
                    nc.scalar.copy(out_sb[:], fin_ps[:])
                    nc.sync.dma_start(out=out_d[b], in_=out_sb[:])
                    if v == 0:
                        yield
                yield

            # drive (finer): overlap pair1 stages into pair0's tail
            g0 = pair_stages(0)
            g1 = pair_stages(1)
            next(g0)           # R0_0
            next(g0)           # R1_0
            next(g0)           # A0_0
            next(g0)           # A1_0 (+sq/colsum/havg/att/cent)
            next(g0)           # B_0
            next(g1)           # R0_1
            next(g0)           # C0_0 (v0 out)
            next(g1)           # R1_1
            next(g0, None)     # C1_0 (v1 out, end)
            next(g1)           # A0_1
            next(g1)           # A1_1
            next(g1)           # B_1
            next(g1)           # C0_1
            next(g1, None)     # C1_1

    if do_compile:
        nc.compile()
    else:
        nc.insert_bir_kernel_barrier_sem_inc()
    return nc


def _get_nc(do_compile=True):
    key = "nc" if do_compile else "nc_sim"
    if key not in _cache:
        _cache[key] = _build(do_compile)
    return _cache[key]


def prepare(inputs, do_compile=True):
    """Build (nc, in_maps) — shared by kernel() and the local sim harness."""
    node_set = np.asarray(inputs["node_set"], dtype=np.float32)
    W0 = np.asarray(inputs["W0"], dtype=np.float32)
    w_i2c = np.asarray(inputs["w_i2c"], dtype=np.float32)
    b_i2c = np.asarray(inputs["b_i2c"], dtype=np.float32)
    W_lin = np.asarray(inputs["W_lin"], dtype=np.float32)
    b_lin = np.asarray(inputs["b_lin"], dtype=np.float32)
    conv_w = np.asarray(inputs["conv_w"], dtype=np.float32)
    conv_b = np.asarray(inputs["conv_b"], dtype=np.float32)
    W_feat = np.asarray(inputs["W_feat"], dtype=np.float32)
    b_feat = np.asarray(inputs["b_feat"], dtype=np.float32)

    i2c_aug = np.ascontiguousarray(
        np.concatenate([w_i2c.T, b_i2c[None, :]], axis=0), dtype=np.float32
    )
    lin_aug = np.ascontiguousarray(
        np.concatenate([W_lin.T, b_lin[None, :]], axis=0), dtype=np.float32
    )
    w0dup = np.ascontiguousarray(np.concatenate([W0, W0], axis=1), dtype=np.float32)
    cw12 = np.ascontiguousarray(np.tile((12.0 * conv_w)[None, :], (128, 1)), dtype=np.float32)
    cb64 = np.full((64, 1), float(conv_b) / 12.0, dtype=np.float32)
    wf_aug = np.ascontiguousarray(
        np.concatenate([W_feat.T, b_feat[None, :]], axis=0), dtype=np.float32
    )
    pp = np.arange(128)
    mm = np.arange(64)
    fold64 = (pp[:, None] % 64 == mm[None, :]).astype(np.float32)
    dup128 = (np.arange(128)[None, :] % 64 == np.arange(64)[:, None]).astype(np.float32)

    nc = _get_nc(do_compile)
    in_maps = []
    for c in range(NCORES):
        in_maps.append({
            "node_set": np.ascontiguousarray(node_set[c * BPC:(c + 1) * BPC]),
            "W0dup": w0dup,
            "i2c_aug": i2c_aug,
            "lin_aug": lin_aug,
            "cw12": cw12,
            "cb64": cb64,
            "wf_aug": wf_aug,
            "fold64": fold64,
            "dup128": dup128,
        })
    return nc, in_maps


def kernel(**inputs):
    from concourse.bass_utils import run_bass_kernel_spmd

    nc, in_maps = prepare(inputs)

    res = run_bass_kernel_spmd(
        nc, in_maps, core_ids=list(range(NCORES)),
        trace=bool(os.environ.get("BASS_TRACE")),
    )
    _cache["last_results"] = res
    out = np.concatenate([res.results[i]["out"] for i in range(NCORES)], axis=0)
    return out



# revision 23
# speedup vs baseline: 1.0037x; 1.0035x over previous
"""Trainium2 Bass kernel for nn_Adaptive_Pooling_Layer (8-core data parallel).

Math (per batch, derived from the reference):
  x = mean_T(node_set)            [N=1024, D=64]   (X12 = 12*x kept unscaled)
  temp = mean_n x; h_avg = tanh(W0^T temp); att = x@h_avg; cent = att@x
  bc1 = relu(w_i2c cent^T + b_i2c);  bc2 = relu(W_lin bc1 + b_lin)  -> [H=8,K=128,D]
  cos[n,h,k] = <x_n, bc_hat_hk> / xn_n    (bc_hat = row-normalized bc2)
  S[n,h] = sum_k cos = <x_n, bsum_h>/xn_n   (bsum_h = sum_k bc_hat_hk)
  C[n,k] = sum_h conv_w[h]*cos/(S+eps) + conv_b
  new[k,d] = sum_n C[n,k] x[n,d];  out = new @ W_feat^T + b_feat

Key restructuring: with Mn[n,h] = conv_w12[h]*inv_xn12[n]/(S[n,h]+eps) and
y_h = Mn[:,h] * x12,  new^T = (1/144) * sum_h (X12^T @ y)_h^T @ bc_hat^T_h
so the [N x HK] cosine matrix is never materialized.

HW notes: PE matmuls with different lhsT partition bases must not target the
same PSUM bank (device fault).  fp32 matmuls emit 2 HW instructions; f32r
(TF32-like, ~1.5e-4 rel) emits 1 and runs 4x, used only on the output path
(z, newT) where the chaotic S-normalizer sensitivity doesn't apply.
"""
import os
import sys

import numpy as np

sys.path.insert(0, "/opt/trn_rl_repo")

B, Din, N, T = 32, 64, 1024, 12
H, K, Dout = 8, 128, 64
NCORES = 8
BPC = B // NCORES  # batches per core
NORM_EPS = 1e-10

_cache = {}

CFG = {"psA": 1, "psB": 1, "psZ": 2, "psS": 3, "nst": 1, "x12": 4,
       "xt2": 4, "bc": 1, "y": 3, "st": 2, "zs": 2, "split_tred": 1}


def _build(do_compile=True):
    import concourse.bass as bass
    import concourse.tile as tile
    from concourse import bacc, masks, mybir

    f32 = mybir.dt.float32
    f32r = mybir.dt.float32r
    AF = mybir.ActivationFunctionType
    ALU = mybir.AluOpType
    AX = mybir.AxisListType

    nc = bacc.Bacc("TRN2", target_bir_lowering=False, debug=False)

    node_d = nc.declare_dram_parameter("node_set", [BPC, Din, N, T], f32, isOutput=False)
    w0_d = nc.declare_dram_parameter("W0dup", [64, 128], f32, isOutput=False)
    i2c_d = nc.declare_dram_parameter("i2c_aug", [2, 8], f32, isOutput=False)
    lin_d = nc.declare_dram_parameter("lin_aug", [9, 1024], f32, isOutput=False)
    cw_d = nc.declare_dram_parameter("cw12", [128, 8], f32, isOutput=False)
    cb_d = nc.declare_dram_parameter("cb64", [64, 1], f32, isOutput=False)
    wf_d = nc.declare_dram_parameter("wf_aug", [65, 64], f32, isOutput=False)
    fold_d = nc.declare_dram_parameter("fold64", [128, 64], f32, isOutput=False)
    dup_d = nc.declare_dram_parameter("dup128", [64, 128], f32, isOutput=False)
    out_d = nc.declare_dram_parameter("out", [BPC, K, Dout], f32, isOutput=True)

    with tile.TileContext(nc) as tc:
        with (
            tc.tile_pool(name="const", bufs=1) as constp,
            tc.tile_pool(name="nst", bufs=CFG["nst"]) as nstp,
            tc.tile_pool(name="xt2", bufs=CFG["xt2"]) as xt2p,
            tc.tile_pool(name="x12", bufs=CFG["x12"]) as x12p,
            tc.tile_pool(name="bc", bufs=CFG["bc"]) as bcp,
            tc.tile_pool(name="y", bufs=CFG["y"]) as yp,
            tc.tile_pool(name="zs", bufs=CFG["zs"]) as zp,
            tc.tile_pool(name="st", bufs=CFG["st"]) as stp,
            tc.tile_pool(name="psA", bufs=CFG["psA"], space="PSUM") as psA,
            tc.tile_pool(name="psB", bufs=CFG["psB"], space="PSUM") as psB,
            tc.tile_pool(name="psZ", bufs=CFG["psZ"], space="PSUM") as psZ,
            tc.tile_pool(name="psS", bufs=CFG["psS"], space="PSUM") as psS,
        ):
            # ---- constants ----
            ident = constp.tile([128, 128], f32)
            masks.make_identity(nc, ident[:])
            w0_sb = constp.tile([64, 128], f32)
            nc.gpsimd.dma_start(out=w0_sb[:], in_=w0_d[:])
            i2c_sb = constp.tile([2, 8], f32)
            nc.gpsimd.dma_start(out=i2c_sb[:], in_=i2c_d[:])
            lin_sb = constp.tile([9, 1024], f32)
            nc.gpsimd.dma_start(out=lin_sb[:], in_=lin_d[:])
            cw_sb = constp.tile([128, 8], f32)
            nc.gpsimd.dma_start(out=cw_sb[:], in_=cw_d[:])
            cb_sb = constp.tile([64, 1], f32)
            nc.gpsimd.dma_start(out=cb_sb[:], in_=cb_d[:])
            wf_sb = constp.tile([65, 64], f32)
            nc.gpsimd.dma_start(out=wf_sb[:], in_=wf_d[:])
            fold_sb = constp.tile([128, 64], f32)
            nc.gpsimd.dma_start(out=fold_sb[:], in_=fold_d[:])
            dup_sb = constp.tile([64, 128], f32)
            nc.gpsimd.dma_start(out=dup_sb[:], in_=dup_d[:])
            eps_sb = constp.tile([128, 1], f32)
            nc.gpsimd.memset(eps_sb[:], 1e-30)

            nsts = []
            for b in range(B // NCORES):
                nst_b = nstp.tile([128, 6144], f32, tag=f"n{b}")
                for q in range(4):
                    p0, n0 = (0, 0) if q < 2 else (64, 512)
                    nq = n0 + (q % 2) * 256
                    nc.sync.dma_start(
                        out=nst_b[p0:p0 + 64, (q % 2) * 3072:(q % 2) * 3072 + 3072]
                            .rearrange("p (n t) -> p n t", t=T),
                        in_=node_d[b, :, nq:nq + 256, :],
                    )
                nsts.append(nst_b)

            def xt2_slice(xT2, i):
                p0 = 0 if i < 4 else 64
                c0 = (i % 4) * 128
                return xT2[p0:p0 + 64, c0:c0 + 128]

            def half(t, i):
                return t[0:64, :] if i < 4 else t[64:128, :]

            def pair_stages(pr):
                # ================= T-sum (both batches) =================
                xT2s = []
                for v in range(2):
                    b = pr * 2 + v
                    nst = nsts[b]
                    xT2 = xt2p.tile([128, 512], f32)
                    for gg in range(2):
                        nc.vector.tensor_reduce(
                            xT2[:, gg * 256:(gg + 1) * 256],
                            nst[:, gg * 3072:(gg + 1) * 3072]
                                .rearrange("p (n t) -> p n t", t=T),
                            axis=AX.X, op=ALU.add,
                        )
                    xT2s.append(xT2)
                    yield

                # ============ transpose to x12 chunks, per batch ============
                x12s, x12rs = [], []
                xn2_2 = stp.tile([128, 16], f32)
                t128_2 = stp.tile([128, 2], f32)
                for v in range(2):
                    xT2 = xT2s[v]
                    xp1 = psA.tile([128, 256], f32, tag="xp1")
                    xp2 = psA.tile([128, 256], f32, tag="xp2")
                    for i in range(8):
                        idn = ident[0:64, 0:64] if i < 4 else ident[64:128, 64:128]
                        xp = xp1 if i < 4 else xp2
                        nc.tensor.transpose(
                            xp[:, (i % 4) * 64:(i % 4 + 1) * 64],
                            xt2_slice(xT2, i), idn,
                        )
                    x12 = x12p.tile([128, 512], f32)
                    nc.scalar.copy(x12[:, 0:256], xp1[:])
                    nc.scalar.copy(x12[:, 256:512], xp2[:])
                    x12r = x12p.tile([128, 512], f32r)
                    nc.scalar.copy(x12r[:, 0:256], xp1[:])
                    nc.scalar.copy(x12r[:, 256:512], xp2[:])
                    x12s.append(x12)
                    x12rs.append(x12r)
                    scr = stp.tile([128, 512], f32)
                    nc.scalar.activation(scr[:], x12[:], AF.Square)
                    nc.vector.tensor_reduce(
                        xn2_2[:, v * 8:(v + 1) * 8],
                        scr[:].rearrange("p (c d) -> p c d", c=8),
                        axis=AX.X, op=ALU.add,
                    )
                    nc.vector.tensor_reduce(
                        t128_2[:, v:v + 1], xT2[:], axis=AX.X, op=ALU.add,
                    )
                    if v == 0:
                        yield
                sq2 = stp.tile([128, 16], f32)
                nc.scalar.activation(sq2[:], xn2_2[:], AF.Sqrt, bias=eps_sb[:])
                inv12_2 = stp.tile([128, 16], f32)  # [n, (v, chunk)] 1/(12 xn)
                nc.vector.reciprocal(inv12_2[:], sq2[:])

                # ========== colsum / temp / h_avg (paired) ==========
                colsum2 = psS.tile([64, 2], f32, tag="small")
                nc.tensor.matmul(colsum2[:], fold_sb[:], t128_2[:], start=True, stop=True)
                tempT2 = stp.tile([64, 2], f32)
                nc.scalar.mul(tempT2[:], colsum2[:], 1.0 / (N * T))
                cbc2 = stp.tile([64, 2], f32)
                nc.scalar.mul(cbc2[:], colsum2[:], cb_sb[:])
                havg_ps2 = psS.tile([128, 2], f32, tag="small")
                nc.tensor.matmul(havg_ps2[:], w0_sb[:], tempT2[:], start=True, stop=True)
                havg2 = stp.tile([128, 2], f32)
                nc.scalar.activation(havg2[:], havg_ps2[:], AF.Tanh)

                # ================== att / cent, per batch ==================
                att2 = stp.tile([128, 16], f32)
                cent_pss = []
                for v in range(2):
                    xT2 = xT2s[v]
                    att_psA = psA.tile([128, 4], f32, tag="xp1")
                    att_psB = psA.tile([128, 4], f32, tag="xp2")
                    for i in range(8):
                        aps = att_psA if i < 4 else att_psB
                        nc.tensor.matmul(
                            aps[:, (i % 4):(i % 4) + 1], xt2_slice(xT2, i),
                            half(havg2, i)[:, v:v + 1], start=True, stop=True,
                        )
                    nc.scalar.copy(att2[:, v * 8:v * 8 + 4], att_psA[:])
                    nc.scalar.copy(att2[:, v * 8 + 4:v * 8 + 8], att_psB[:])
                    cent_ps = psS.tile([1, 64], f32, tag="small")
                    for i in range(8):
                        nc.tensor.matmul(
                            cent_ps[:], att2[:, v * 8 + i:v * 8 + i + 1],
                            x12s[v][:, i * 64:(i + 1) * 64],
                            start=(i == 0), stop=(i == 7),
                        )
                    cent_pss.append(cent_ps)
                yield

                # ============ centroid generator (paired) ============
                cent_aug2 = stp.tile([2, 128], f32)
                nc.gpsimd.memset(cent_aug2[:], 1.0)
                for v in range(2):
                    nc.scalar.mul(
                        cent_aug2[0:1, v * 64:(v + 1) * 64], cent_pss[v][:],
                        1.0 / 144.0,
                    )
                bc1_ps2 = psS.tile([8, 128], f32, tag="small")
                nc.tensor.matmul(bc1_ps2[:], i2c_sb[:], cent_aug2[:], start=True, stop=True)
                bc1_aug2 = stp.tile([9, 128], f32)
                nc.gpsimd.memset(bc1_aug2[:], 1.0)
                nc.scalar.activation(bc1_aug2[0:8, :], bc1_ps2[:], AF.Relu)

                # bcr2: [k, (v, h, d)] relu'd bc2 for both batches
                bcr2 = bcp.tile([128, 1024], f32)
                bcr2_v = bcr2[:].rearrange("p (v h d) -> p v h d", v=2, h=8)
                for h in range(8):
                    bc2_ps = psS.tile([128, 128], f32, tag="small")
                    nc.tensor.matmul(
                        bc2_ps[:], lin_sb[:, h * 128:(h + 1) * 128], bc1_aug2[:],
                        start=True, stop=True,
                    )
                    nc.scalar.activation(
                        bcr2_v[:, :, h, :],
                        bc2_ps[:].rearrange("p (v d) -> p v d", v=2), AF.Relu,
                    )
                scrc = stp.tile([128, 1024], f32)
                nc.vector.scalar_tensor_tensor(
                    out=scrc[:], in0=bcr2[:], scalar=0.0, in1=bcr2[:],
                    op0=ALU.bypass, op1=ALU.mult,
                )
                cn2_2 = stp.tile([128, 16], f32)
                nc.vector.tensor_reduce(
                    cn2_2[:], scrc[:].rearrange("p (g d) -> p g d", g=16),
                    axis=AX.X, op=ALU.add,
                )
                sqc2 = stp.tile([128, 16], f32)
                nc.scalar.activation(sqc2[:], cn2_2[:], AF.Sqrt, bias=eps_sb[:])
                invcn2 = stp.tile([128, 16], f32)
                nc.vector.reciprocal(invcn2[:], sqc2[:])
                bchat2 = bcp.tile([128, 1024], f32)  # [k, (v, h, d)] normalized
                nc.vector.scalar_tensor_tensor(
                    out=bchat2[:].rearrange("p (g d) -> p g d", g=16),
                    in0=bcr2[:].rearrange("p (g d) -> p g d", g=16),
                    scalar=0.0,
                    in1=invcn2[:].rearrange("p (g d) -> p g d", d=1)
                        .broadcast_to([128, 16, 64]),
                    op0=ALU.bypass, op1=ALU.mult,
                )

                # ====== bc_hat transposes -> bchT2 [d, (v, h, k)] ======
                bchT2 = bcp.tile([64, 2048], f32)
                bchTr2 = bcp.tile([64, 2048], f32r)
                for g in range(4):
                    bchT_ps = psB.tile([64, 512], f32, tag="bchT")
                    for j in range(4):
                        hh = g * 4 + j  # global (v, h) index
                        nc.tensor.transpose(
                            bchT_ps[:, j * 128:(j + 1) * 128],
                            bchat2[:, hh * 64:(hh + 1) * 64], ident[:],
                        )
                    nc.scalar.copy(bchT2[:, g * 512:(g + 1) * 512], bchT_ps[:])
                    nc.scalar.copy(bchTr2[:, g * 512:(g + 1) * 512], bchT_ps[:])

                # ====== bsum (both batches) + duplication matmul ======
                bsum2 = stp.tile([64, 16], f32)
                nc.vector.tensor_reduce(
                    bsum2[:], bchT2[:].rearrange("p (g k) -> p g k", g=16),
                    axis=AX.X, op=ALU.add,
                )
                bsum_ps2 = psS.tile([128, 16], f32, tag="small")
                nc.tensor.matmul(bsum_ps2[:], dup_sb[:], bsum2[:], start=True, stop=True)
                bsumT2 = stp.tile([128, 16], f32)  # [(dup), (v, h)]
                nc.scalar.copy(bsumT2[:], bsum_ps2[:])
                yield

                # ============ S, E/Mn, y, z, newT, out per batch ============
                for v in range(2):
                    b = pr * 2 + v
                    xT2 = xT2s[v]
                    x12 = x12s[v]
                    x12r = x12rs[v]
                    S_A = psA.tile([128, 32], f32, tag="xp1")
                    S_B = psA.tile([128, 32], f32, tag="xp2")
                    for i in range(8):
                        sp = S_A if i < 4 else S_B
                        nc.tensor.matmul(
                            sp[:, (i % 4) * 8:(i % 4 + 1) * 8], xt2_slice(xT2, i),
                            half(bsumT2, i)[:, v * 8:(v + 1) * 8],
                            start=True, stop=True,
                        )
                    Mn = stp.tile([128, 64], f32)  # [n, (chunk, h)]
                    for g, sp in enumerate((S_A, S_B)):
                        ms = Mn[:, g * 32:(g + 1) * 32]
                        iv = (
                            inv12_2[:, v * 8 + g * 4:v * 8 + (g + 1) * 4]
                            .rearrange("p (c h) -> p c h", h=1)
                            .broadcast_to([128, 4, 8])
                        )
                        msv = ms.rearrange("p (c h) -> p c h", c=4)
                        nc.vector.scalar_tensor_tensor(
                            out=msv, in0=sp[:].rearrange("p (c h) -> p c h", c=4),
                            scalar=0.0, in1=iv, op0=ALU.bypass, op1=ALU.mult,
                        )
                        nc.vector.tensor_scalar(
                            out=ms, in0=ms, scalar1=NORM_EPS, scalar2=None,
                            op0=ALU.add,
                        )
                        nc.vector.reciprocal(ms, ms)
                        nc.vector.scalar_tensor_tensor(
                            out=msv, in0=msv, scalar=0.0, in1=iv,
                            op0=ALU.bypass, op1=ALU.mult,
                        )
                        nc.vector.scalar_tensor_tensor(
                            out=msv, in0=msv, scalar=0.0,
                            in1=cw_sb[:].rearrange("p (c h) -> p c h", c=1)
                                .broadcast_to([128, 4, 8]),
                            op0=ALU.bypass, op1=ALU.mult,
                        )

                    z_ps = psZ.tile([64, 512], f32, tag="zz")
                    for i in range(8):
                        yt = yp.tile([128, 512], f32r)
                        nc.vector.scalar_tensor_tensor(
                            out=yt[:].rearrange("p (h d) -> p h d", h=8),
                            in0=x12[:, i * 64:(i + 1) * 64]
                                .rearrange("p (h d) -> p h d", h=1)
                                .broadcast_to([128, 8, 64]),
                            scalar=0.0,
                            in1=Mn[:, i * 8:(i + 1) * 8]
                                .rearrange("p (h d) -> p h d", d=1)
                                .broadcast_to([128, 8, 64]),
                            op0=ALU.bypass, op1=ALU.mult,
                        )
                        nc.tensor.matmul(
                            z_ps[:], x12r[:, i * 64:(i + 1) * 64], yt[:],
                            start=(i == 0), stop=(i == 7),
                        )
                    z_sb = zp.tile([64, 512], f32r)
                    nc.scalar.mul(z_sb[:], z_ps[:], 1.0 / 144.0)

                    newT_ps = psZ.tile([64, 128], f32, tag="zz")
                    for h in range(8):
                        nc.tensor.matmul(
                            newT_ps[:],
                            z_sb[:, h * 64:(h + 1) * 64],
                            bchTr2[:, (v * 8 + h) * 128:(v * 8 + h + 1) * 128],
                            start=(h == 0), stop=(h == 7),
                        )
                    newT_aug = stp.tile([65, 128], f32)
                    nc.gpsimd.memset(newT_aug[:], 1.0)
                    nc.vector.tensor_scalar(
                        out=newT_aug[0:64, :], in0=newT_ps[:],
                        scalar1=cbc2[:, v:v + 1], scalar2=None, op0=ALU.add,
                    )
                    fin_ps = psS.tile([128, 64], f32, tag="small")
                    nc.tensor.matmul(fin_ps[:], newT_aug[:], wf_sb[:],
                                     start=True, stop=True)
                    out_sb = stp.tile([128, 64], f32)
                    nc.scalar.copy(out_sb[:], fin_ps[:])
                    nc.sync.dma_start(out=out_d[b], in_=out_sb[:])
                    if v == 0:
                        yield
                yield

            # drive (finer): overlap pair1 stages into pair0's tail
            g0 = pair_stages(0)
            g1 = pair_stages(1)
            next(g0)           # R0_0
            next(g0)           # R1_0
            next(g0)           # A0_0
            next(g0)           # A1_0 (+sq/colsum/havg/att/cent)
            next(g0)           # B_0
            next(g1)           # R0_1
            next(g0)           # C0_0 (v0 out)
            next(g1)           # R1_1
            next(g0, None)     # C1_0 (v1 out, end)
            next(g1)           # A0_1
            next(g1)           # A1_1
            next(g1)           # B_1
            next(g1)           # C0_1
            next(g1, None)     # C1_1

    if do_compile:
        nc.compile()
    else:
        nc.insert_bir_kernel_barrier_sem_inc()
    return nc


def _get_nc(do_compile=True):
    key = "nc" if do_compile else "nc_sim"
    if key not in _cache:
        _cache[key] = _build(do_compile)
    return _cache[key]


def prepare(inputs, do_compile=True):
    """Build (nc, in_maps) — shared by kernel() and the local sim harness."""
    node_set = np.asarray(inputs["node_set"], dtype=np.float32)
    W0 = np.asarray(inputs["W0"], dtype=np.float32)
    w_i2c = np.asarray(inputs["w_i2c"], dtype=np.float32)
    b_i2c = np.asarray(inputs["b_i2c"], dtype=np.float32)
    W_lin = np.asarray(inputs["W_lin"], dtype=np.float32)
    b_lin = np.asarray(inputs["b_lin"], dtype=np.float32)
    conv_w = np.asarray(inputs["conv_w"], dtype=np.float32)
    conv_b = np.asarray(inputs["conv_b"], dtype=np.float32)
    W_feat = np.asarray(inputs["W_feat"], dtype=np.float32)
    b_feat = np.asarray(inputs["b_feat"], dtype=np.float32)

    i2c_aug = np.ascontiguousarray(
        np.concatenate([w_i2c.T, b_i2c[None, :]], axis=0), dtype=np.float32
    )
    lin_aug = np.ascontiguousarray(
        np.concatenate([W_lin.T, b_lin[None, :]], axis=0), dtype=np.float32
    )
    w0dup = np.ascontiguousarray(np.concatenate([W0, W0], axis=1), dtype=np.float32)
    cw12 = np.ascontiguousarray(np.tile((12.0 * conv_w)[None, :], (128, 1)), dtype=np.float32)
    cb64 = np.full((64, 1), float(conv_b) / 12.0, dtype=np.float32)
    wf_aug = np.ascontiguousarray(
        np.concatenate([W_feat.T, b_feat[None, :]], axis=0), dtype=np.float32
    )
    pp = np.arange(128)
    mm = np.arange(64)
    fold64 = (pp[:, None] % 64 == mm[None, :]).astype(np.float32)
    dup128 = (np.arange(128)[None, :] % 64 == np.arange(64)[:, None]).astype(np.float32)

    nc = _get_nc(do_compile)
    in_maps = []
    for c in range(NCORES):
        in_maps.append({
            "node_set": np.ascontiguousarray(node_set[c * BPC:(c + 1) * BPC]),
            "W0dup": w0dup,
            "i2c_aug": i2c_aug,
            "lin_aug": lin_aug,
            "cw12": cw12,
            "cb64": cb64,
            "wf_aug": wf_aug,
            "fold64": fold64,
            "dup128": dup128,
        })
    return nc, in_maps


def kernel(**inputs):
    from concourse.bass_utils import run_bass_kernel_spmd

    nc, in_maps = prepare(inputs)

    res = run_bass_kernel_spmd(
        nc, in_maps, core_ids=list(range(NCORES)),
        trace=bool(os.environ.get("BASS_TRACE")),
    )
    _cache["last_results"] = res
    out = np.concatenate([res.results[i]["out"] for i in range(NCORES)], axis=0)
    return out

